# revision 3
# baseline (speedup 1.0000x reference)
"""Upfirdn2d-style blur kernel for Trainium2 (Bass/Tile), 8-core SPMD.

Computes: zero-insertion 2x upsample + pad(2,1,2,1) + depthwise 4x4 FIR
  filter outer([1,3,3,1],[1,3,3,1])/64 * 4  (separable, symmetric)
on x of shape (16, 512, 32, 32) f32 -> (16, 512, 64, 64) f32.

Polyphase separable decomposition (verified vs reference, ~1e-7 abs err):
  vertical  : t[2r]   = (3*x[r] + x[r-1])/16 ; t[2r+1] = (3*x[r] + x[r+1])/16
  horizontal: o[2c]   = 3*t[c] + t[c-1]      ; o[2c+1] = 3*t[c] + t[c+1]
(out-of-range x/t taps are zero)

Sharding: pure data parallel over the 8192 independent images (batch*channel,
conv is depthwise) -> 1024 images per core, no cross-core communication.

The pass is DMA-bandwidth-bound: 4 MiB in + 16 MiB out per core at the
~360 GB/s per-core DMA roofline = 58.2 us. The kernel is structured so the
DMA bus never idles:

  - one 32x32 image per partition-row, 8 iterations of 128 images;
  - all 8 input DMAs are issued back-to-back at t=0 on the SP (sync) HWDGE
    queue ahead of the output DMAs (the SP ring is FIFO, so inputs stream
    first while iteration-0 compute runs);
  - compute is split across three engines so no engine exceeds ~60% of the
    DMA time: ACT does the two prescales (x*3/16, x*1/16), Pool does the
    vertical pass as two plain adds, DVE does the horizontal pass as two
    fused scalar_tensor_tensor ops;
  - zero "guard" rows/cols (x/16 stored in a 34x32 tile with zero rows at
    both ends; t stored 64x34 with zero cols) make every boundary tap exact
    with no special-case instructions. Guards are zeroed once per pool slot
    (slots recycle; interior writes never touch them);
  - the first 3 iterations are split into row-halves so the first output
    DMA is ready the moment the input stream drains (~14 us), leaving no
    bus gap;
  - walrus accepts only one sync-wait per instruction, so extra waits are
    hoisted onto same-engine NoOps (_split_multi_waits).

TimelineSim: single pass 62.2 us (= 2.3 us ramp + 58.2 us DMA floor +
1.7 us drain), steady-state 58.2 us/pass; previous version simulated 73.6.
"""

import numpy as np

import concourse.bass as bass
import concourse.mybir as mybir
import concourse.tile as tile
from concourse.bass_utils import run_bass_kernel_spmd

N_CORES = 8
B, C, H, W = 16, 512, 32, 32
IMGS = B * C                  # 8192 independent images
PER_CORE = IMGS // N_CORES    # 1024
P = 128                       # SBUF partitions
SUB = 1                       # images per partition per iteration
N_ITERS = PER_CORE // (P * SUB)   # 8
IMG = H * W                   # 1024 elems per input image
OIMG = 4 * IMG                # 4096 elems per output image

F32 = mybir.dt.float32
A = mybir.AluOpType


def _split_multi_waits(nc: bass.Bass) -> None:
    """walrus rejects >1 sync-wait per instruction; hoist extras onto NoOps.

    A NoOp on the same engine queue immediately before the instruction
    executes its wait first, so splitting the AND-list of waits across a
    NoOp chain is semantically identical.
    """
    for fn in nc.m.functions:
        for bb in fn.blocks:
            insts = bb.instructions
            i = 0
            while i < len(insts):
                inst = insts[i]
                si = inst.sync_info
                if si is not None and len(si.on_wait) > 1:
                    waits = list(si.on_wait)
                    for j, w in enumerate(waits[:-1]):
                        nop = mybir.InstNoOp(
                            name=nc.get_next_instruction_name(),
                            text_hint=f"wait_split_{j}")
                        nop.engine = inst.engine
                        nop.sync_info = mybir.SyncInfo(
                            on_wait=[w], on_update=[])
                        insts.insert(i, nop)
                        i += 1
                    inst.sync_info = mybir.SyncInfo(
                        on_wait=[waits[-1]], on_update=list(si.on_update))
                i += 1


def build_nc(repeat: int = 1, sub: int = SUB, in_q: str = "sync",
             out_q: str = "sync", pin_bufs: int = 0, po_bufs: int = 3,
             pt_bufs: int = 3, px_bufs: int = 3,
             split_waits: bool = True, hoist_in: bool = True,
             first_split: int = 2, first_split_iters: int = 4,
             guard_once: bool = True,
             tguard_eng: str = "scalar") -> bass.Bass:
    nc = bass.Bass()
    x = nc.dram_tensor("x", (PER_CORE, IMG), F32, kind="ExternalInput")
    out = nc.dram_tensor("out", (PER_CORE, OIMG), F32, kind="ExternalOutput")
    in_dma = getattr(nc, in_q)
    out_dma = getattr(nc, out_q)

    n_iters = PER_CORE // (P * sub)
    if not pin_bufs:
        pin_bufs = n_iters
    XQ = 34 * 32            # guarded x/16 image (rows -1..32)
    TG = 64 * 34            # guarded t image (cols -1..32)
    with tile.TileContext(nc) as tc:
        with (
            tc.tile_pool(name="pin", bufs=pin_bufs) as pin,
            tc.tile_pool(name="px3", bufs=px_bufs) as px3,
            tc.tile_pool(name="pxq", bufs=px_bufs) as pxq,
            tc.tile_pool(name="pt", bufs=pt_bufs) as pt,
            tc.tile_pool(name="po", bufs=po_bufs) as po,
        ):
            def issue_in(i):
                base = i * P * sub
                xin = pin.tile([P, sub * IMG], F32, tag="xin")
                x_dram = bass.AP(x, base * IMG,
                                 [[IMG, P], [P * IMG, sub], [1, IMG]])
                in_dma.dma_start(
                    out=xin[:].rearrange("p (s c) -> p s c", s=sub),
                    in_=x_dram)
                return xin

            pending = {}
            for it in range(repeat * n_iters):
                i = it % n_iters
                base = i * P * sub

                if hoist_in:
                    if it % n_iters == 0:
                        for j in range(n_iters):
                            pending[j] = issue_in(j)
                    xin = pending.pop(i)
                else:
                    xin = issue_in(i)

                x3 = px3.tile([P, sub * IMG], F32, tag="x3")
                xq = pxq.tile([P, sub * XQ], F32, tag="xq")
                t = pt.tile([P, sub * TG], F32, tag="t")
                o = po.tile([P, sub * OIMG], F32, tag="o")
                xin_a, x3_a = xin[:], x3[:]
                xq_a, t_a, o_a = xq[:], t[:], o[:]

                def ap(h, off, dims):
                    return bass.AP(h.tensor, h.offset + off, [h.ap[0]] + dims)

                # ACT: x3 = x*(3/16)  (contiguous)
                nc.scalar.mul(x3_a, xin_a, 3.0 / 16.0)
                # ACT: xq_g rows 1..32 = x*(1/16)
                nc.scalar.mul(
                    ap(xq_a, 32, [[XQ, sub], [1, IMG]]),
                    ap(xin_a, 0, [[IMG, sub], [1, IMG]]),
                    1.0 / 16.0)
                # xq_g guard rows {0,33} = 0  (0 * finite xin)
                if not guard_once or it < px_bufs:
                    nc.scalar.mul(
                        ap(xq_a, 0, [[XQ, sub], [33 * 32, 2], [1, 32]]),
                        ap(xin_a, 0, [[IMG, sub], [32, 2], [1, 32]]),
                        0.0)

                # t_g guard cols {0,33} = 0
                if not guard_once or it < pt_bufs:
                    if tguard_eng == "gpsimd":
                        nc.gpsimd.memset(
                            ap(t_a, 0, [[TG, sub], [34, 64], [33, 2]]), 0.0)
                    else:
                        nc.scalar.mul(
                            ap(t_a, 0, [[TG, sub], [34, 64], [33, 2]]),
                            ap(xin_a, 0, [[IMG, sub], [8, 64], [1, 2]]),
                            0.0)

                pieces = first_split if (i < first_split_iters
                                         and sub == 1) else 1
                for pc in range(pieces):
                    r0 = 32 * pc // pieces      # input-row range of piece
                    rn = 32 // pieces
                    # Pool: t_g even rows 2r, interior = x3[r] + xq_g[r]
                    nc.gpsimd.tensor_tensor(
                        ap(t_a, 2 * r0 * 34 + 1,
                           [[TG, sub], [68, rn], [1, 32]]),
                        ap(x3_a, r0 * 32, [[IMG, sub], [32, rn], [1, 32]]),
                        ap(xq_a, r0 * 32, [[XQ, sub], [32, rn], [1, 32]]),
                        A.add)
                    # Pool: t_g odd rows 2r+1 = x3[r] + xq_g[r+2]
                    nc.gpsimd.tensor_tensor(
                        ap(t_a, (2 * r0 + 1) * 34 + 1,
                           [[TG, sub], [68, rn], [1, 32]]),
                        ap(x3_a, r0 * 32, [[IMG, sub], [32, rn], [1, 32]]),
                        ap(xq_a, (r0 + 2) * 32,
                           [[XQ, sub], [32, rn], [1, 32]]),
                        A.add)
                    # DVE: o rows 2r0..2r0+2rn, even cols = 3*t_g[c+1]+t_g[c]
                    nc.vector.scalar_tensor_tensor(
                        ap(o_a, 2 * r0 * 64,
                           [[OIMG, sub], [64, 2 * rn], [2, 32]]),
                        ap(t_a, 2 * r0 * 34 + 1,
                           [[TG, sub], [34, 2 * rn], [1, 32]]),
                        3.0,
                        ap(t_a, 2 * r0 * 34,
                           [[TG, sub], [34, 2 * rn], [1, 32]]),
                        A.mult, A.add)
                    # DVE: odd cols = 3*t_g[c+1] + t_g[c+2]
                    nc.vector.scalar_tensor_tensor(
                        ap(o_a, 2 * r0 * 64 + 1,
                           [[OIMG, sub], [64, 2 * rn], [2, 32]]),
                        ap(t_a, 2 * r0 * 34 + 1,
                           [[TG, sub], [34, 2 * rn], [1, 32]]),
                        3.0,
                        ap(t_a, 2 * r0 * 34 + 2,
                           [[TG, sub], [34, 2 * rn], [1, 32]]),
                        A.mult, A.add)

                    o_dram = bass.AP(
                        out, base * OIMG + 2 * r0 * 64,
                        [[OIMG, P], [P * OIMG, sub], [1, 2 * rn * 64]])
                    out_dma.dma_start(
                        out=o_dram,
                        in_=ap(o_a, 2 * r0 * 64,
                               [[OIMG, sub], [1, 2 * rn * 64]]))
    if split_waits:
        _split_multi_waits(nc)
    return nc


def kernel(x: np.ndarray) -> np.ndarray:
    x = np.ascontiguousarray(np.asarray(x), dtype=np.float32)
    assert x.shape == (B, C, H, W), x.shape
    flat = x.reshape(IMGS, IMG)
    in_maps = [
        {"x": flat[c * PER_CORE:(c + 1) * PER_CORE]} for c in range(N_CORES)
    ]
    nc = build_nc()
    res = run_bass_kernel_spmd(nc, in_maps, core_ids=list(range(N_CORES)))
    outs = [res.results[c]["out"] for c in range(N_CORES)]
    full = np.concatenate(outs, axis=0).reshape(B, C, 2 * H, 2 * W)
    return full


if __name__ == "__main__":
    rng = np.random.default_rng(0)
    xt = rng.standard_normal((B, C, H, W), dtype=np.float32)
    yt = kernel(xt)
    print("out", yt.shape, yt.dtype)


# revision 4
# speedup vs baseline: 1.0243x; 1.0243x over previous
"""Upfirdn2d-style blur kernel for Trainium2 (Bass/Tile), 8-core SPMD.

Computes: zero-insertion 2x upsample + pad(2,1,2,1) + depthwise 4x4 FIR
  filter outer([1,3,3,1],[1,3,3,1])/64 * 4  (separable, symmetric)
on x of shape (16, 512, 32, 32) f32 -> (16, 512, 64, 64) f32.

Polyphase separable decomposition (verified vs reference, ~1e-7 abs err):
  vertical  : t[2r]   = (3*x[r] + x[r-1])/16 ; t[2r+1] = (3*x[r] + x[r+1])/16
  horizontal: o[2c]   = 3*t[c] + t[c-1]      ; o[2c+1] = 3*t[c] + t[c+1]
(out-of-range x/t taps are zero)

Sharding: pure data parallel over the 8192 independent images (batch*channel,
conv is depthwise) -> 1024 images per core, no cross-core communication.

The pass is DMA-bandwidth-bound: 4 MiB in + 16 MiB out per core at the
~360 GB/s per-core DMA roofline = 58.2 us. The kernel is structured so the
DMA bus never idles:

  - one 32x32 image per partition-row, 8 iterations of 128 images;
  - all 8 input DMAs are issued back-to-back at t=0 on the SP (sync) HWDGE
    queue ahead of the output DMAs (the SP ring is FIFO, so inputs stream
    first while iteration-0 compute runs);
  - compute is split across three engines so no engine exceeds ~60% of the
    DMA time: ACT does the two prescales (x*3/16, x*1/16), Pool does the
    vertical pass as two plain adds, DVE does the horizontal pass as two
    fused scalar_tensor_tensor ops;
  - zero "guard" rows/cols (x/16 stored in a 34x32 tile with zero rows at
    both ends; t stored 64x34 with zero cols) make every boundary tap exact
    with no special-case instructions. Guards are zeroed once per pool slot
    (slots recycle; interior writes never touch them);
  - the first 4 iterations are split into row-halves so the first output
    DMA is ready the moment the input stream drains (~14 us), leaving no
    bus gap;
  - walrus accepts only one sync-wait per instruction, so extra waits are
    hoisted onto same-engine NoOps (_split_multi_waits).

TimelineSim: single pass 62.2 us (= 2.3 us ramp + 58.2 us DMA floor +
1.7 us drain), steady-state 58.2 us/pass; previous version simulated 73.6.
"""

import numpy as np

import concourse.bass as bass
import concourse.mybir as mybir
import concourse.tile as tile
from concourse.bass_utils import run_bass_kernel_spmd

N_CORES = 8
B, C, H, W = 16, 512, 32, 32
IMGS = B * C                  # 8192 independent images
PER_CORE = IMGS // N_CORES    # 1024
P = 128                       # SBUF partitions
SUB = 1                       # images per partition per iteration
N_ITERS = PER_CORE // (P * SUB)   # 8
IMG = H * W                   # 1024 elems per input image
OIMG = 4 * IMG                # 4096 elems per output image

F32 = mybir.dt.float32
A = mybir.AluOpType


def _split_multi_waits(nc: bass.Bass) -> None:
    """walrus rejects >1 sync-wait per instruction; hoist extras onto NoOps.

    A NoOp on the same engine queue immediately before the instruction
    executes its wait first, so splitting the AND-list of waits across a
    NoOp chain is semantically identical.
    """
    for fn in nc.m.functions:
        for bb in fn.blocks:
            insts = bb.instructions
            i = 0
            while i < len(insts):
                inst = insts[i]
                si = inst.sync_info
                if si is not None and len(si.on_wait) > 1:
                    waits = list(si.on_wait)
                    for j, w in enumerate(waits[:-1]):
                        nop = mybir.InstNoOp(
                            name=nc.get_next_instruction_name(),
                            text_hint=f"wait_split_{j}")
                        nop.engine = inst.engine
                        nop.sync_info = mybir.SyncInfo(
                            on_wait=[w], on_update=[])
                        insts.insert(i, nop)
                        i += 1
                    inst.sync_info = mybir.SyncInfo(
                        on_wait=[waits[-1]], on_update=list(si.on_update))
                i += 1


def build_nc(repeat: int = 1, sub: int = SUB, in_q: str = "sync",
             out_q: str = "sync", pin_bufs: int = 0, po_bufs: int = 3,
             pt_bufs: int = 3, px_bufs: int = 3,
             split_waits: bool = True, hoist_in: bool = True,
             first_split: int = 2, first_split_iters: int = 4,
             guard_once: bool = True,
             tguard_eng: str = "scalar") -> bass.Bass:
    nc = bass.Bass()
    x = nc.dram_tensor("x", (PER_CORE, IMG), F32, kind="ExternalInput")
    out = nc.dram_tensor("out", (PER_CORE, OIMG), F32, kind="ExternalOutput")
    in_dma = getattr(nc, in_q)
    out_dma = getattr(nc, out_q)

    n_iters = PER_CORE // (P * sub)
    if not pin_bufs:
        pin_bufs = n_iters
    XQ = 34 * 32            # guarded x/16 image (rows -1..32)
    TG = 64 * 34            # guarded t image (cols -1..32)
    with tile.TileContext(nc) as tc:
        with (
            tc.tile_pool(name="pin", bufs=pin_bufs) as pin,
            tc.tile_pool(name="px3", bufs=px_bufs) as px3,
            tc.tile_pool(name="pxq", bufs=px_bufs) as pxq,
            tc.tile_pool(name="pt", bufs=pt_bufs) as pt,
            tc.tile_pool(name="po", bufs=po_bufs) as po,
        ):
            def issue_in(i):
                base = i * P * sub
                xin = pin.tile([P, sub * IMG], F32, tag="xin")
                x_dram = bass.AP(x, base * IMG,
                                 [[IMG, P], [P * IMG, sub], [1, IMG]])
                in_dma.dma_start(
                    out=xin[:].rearrange("p (s c) -> p s c", s=sub),
                    in_=x_dram)
                return xin

            pending = {}
            for it in range(repeat * n_iters):
                i = it % n_iters
                base = i * P * sub

                if hoist_in:
                    if it % n_iters == 0:
                        for j in range(n_iters):
                            pending[j] = issue_in(j)
                    xin = pending.pop(i)
                else:
                    xin = issue_in(i)

                x3 = px3.tile([P, sub * IMG], F32, tag="x3")
                xq = pxq.tile([P, sub * XQ], F32, tag="xq")
                t = pt.tile([P, sub * TG], F32, tag="t")
                o = po.tile([P, sub * OIMG], F32, tag="o")
                xin_a, x3_a = xin[:], x3[:]
                xq_a, t_a, o_a = xq[:], t[:], o[:]

                def ap(h, off, dims):
                    return bass.AP(h.tensor, h.offset + off, [h.ap[0]] + dims)

                # ACT: x3 = x*(3/16)  (contiguous)
                nc.scalar.mul(x3_a, xin_a, 3.0 / 16.0)
                # ACT: xq_g rows 1..32 = x*(1/16)
                nc.scalar.mul(
                    ap(xq_a, 32, [[XQ, sub], [1, IMG]]),
                    ap(xin_a, 0, [[IMG, sub], [1, IMG]]),
                    1.0 / 16.0)
                # xq_g guard rows {0,33} = 0  (0 * finite xin)
                if not guard_once or it < px_bufs:
                    nc.scalar.mul(
                        ap(xq_a, 0, [[XQ, sub], [33 * 32, 2], [1, 32]]),
                        ap(xin_a, 0, [[IMG, sub], [32, 2], [1, 32]]),
                        0.0)

                # t_g guard cols {0,33} = 0
                if not guard_once or it < pt_bufs:
                    if tguard_eng == "gpsimd":
                        nc.gpsimd.memset(
                            ap(t_a, 0, [[TG, sub], [34, 64], [33, 2]]), 0.0)
                    else:
                        nc.scalar.mul(
                            ap(t_a, 0, [[TG, sub], [34, 64], [33, 2]]),
                            ap(xin_a, 0, [[IMG, sub], [8, 64], [1, 2]]),
                            0.0)

                pieces = first_split if (i < first_split_iters
                                         and sub == 1) else 1
                for pc in range(pieces):
                    r0 = 32 * pc // pieces      # input-row range of piece
                    rn = 32 // pieces
                    # Pool: t_g even rows 2r, interior = x3[r] + xq_g[r]
                    nc.gpsimd.tensor_tensor(
                        ap(t_a, 2 * r0 * 34 + 1,
                           [[TG, sub], [68, rn], [1, 32]]),
                        ap(x3_a, r0 * 32, [[IMG, sub], [32, rn], [1, 32]]),
                        ap(xq_a, r0 * 32, [[XQ, sub], [32, rn], [1, 32]]),
                        A.add)
                    # Pool: t_g odd rows 2r+1 = x3[r] + xq_g[r+2]
                    nc.gpsimd.tensor_tensor(
                        ap(t_a, (2 * r0 + 1) * 34 + 1,
                           [[TG, sub], [68, rn], [1, 32]]),
                        ap(x3_a, r0 * 32, [[IMG, sub], [32, rn], [1, 32]]),
                        ap(xq_a, (r0 + 2) * 32,
                           [[XQ, sub], [32, rn], [1, 32]]),
                        A.add)
                    # DVE: o rows 2r0..2r0+2rn, even cols = 3*t_g[c+1]+t_g[c]
                    nc.vector.scalar_tensor_tensor(
                        ap(o_a, 2 * r0 * 64,
                           [[OIMG, sub], [64, 2 * rn], [2, 32]]),
                        ap(t_a, 2 * r0 * 34 + 1,
                           [[TG, sub], [34, 2 * rn], [1, 32]]),
                        3.0,
                        ap(t_a, 2 * r0 * 34,
                           [[TG, sub], [34, 2 * rn], [1, 32]]),
                        A.mult, A.add)
                    # DVE: odd cols = 3*t_g[c+1] + t_g[c+2]
                    nc.vector.scalar_tensor_tensor(
                        ap(o_a, 2 * r0 * 64 + 1,
                           [[OIMG, sub], [64, 2 * rn], [2, 32]]),
                        ap(t_a, 2 * r0 * 34 + 1,
                           [[TG, sub], [34, 2 * rn], [1, 32]]),
                        3.0,
                        ap(t_a, 2 * r0 * 34 + 2,
                           [[TG, sub], [34, 2 * rn], [1, 32]]),
                        A.mult, A.add)

                    o_dram = bass.AP(
                        out, base * OIMG + 2 * r0 * 64,
                        [[OIMG, P], [P * OIMG, sub], [1, 2 * rn * 64]])
                    out_dma.dma_start(
                        out=o_dram,
                        in_=ap(o_a, 2 * r0 * 64,
                               [[OIMG, sub], [1, 2 * rn * 64]]))
    if split_waits:
        _split_multi_waits(nc)
    return nc


def kernel(x: np.ndarray) -> np.ndarray:
    x = np.ascontiguousarray(np.asarray(x), dtype=np.float32)
    assert x.shape == (B, C, H, W), x.shape
    flat = x.reshape(IMGS, IMG)
    in_maps = [
        {"x": flat[c * PER_CORE:(c + 1) * PER_CORE]} for c in range(N_CORES)
    ]
    nc = build_nc()
    res = run_bass_kernel_spmd(nc, in_maps, core_ids=list(range(N_CORES)))
    outs = [res.results[c]["out"] for c in range(N_CORES)]
    full = np.concatenate(outs, axis=0).reshape(B, C, 2 * H, 2 * W)
    return full


if __name__ == "__main__":
    rng = np.random.default_rng(0)
    xt = rng.standard_normal((B, C, H, W), dtype=np.float32)
    yt = kernel(xt)
    print("out", yt.shape, yt.dtype)
